# revision 1
# baseline (speedup 1.0000x reference)
"""Trainium2 Bass kernel for nn_CustomerizedLoss (MSE + per-sample weight-conditioned
MLP cross-entropy over a fixed image set).

Sharding: model-batch dim B=64 split across 8 NeuronCores (8 samples each);
the 10000x784 image matrix is replicated (shipped transposed, bf16).

Per core:
  mm1:  h^T[bh=512, n] = W1T[785, 512]^T @ imagesT_ext[785, n]   (bias via ones-row)
  relu: ScalarE psum->sbuf bf16
  mm2:  logits[n, 80] = h^T^T @ W2blk[512, 80] + ones-row @ B2   (block-diag W2)
  CE:   grouped (8 groups of 10) log-softmax + one-hot label dot, accumulated
  loss1: sum((inp1-tar1)^2) over this core's 8 rows
Host combines partial sums into (combined, loss1, loss2).
"""

import numpy as np
import ml_dtypes

BF16 = ml_dtypes.bfloat16
FP8 = ml_dtypes.float8_e4m3

INPUT, HIDDEN, OUT = 784, 64, 10
NTEST, B, WVEC = 10000, 64, 50890
NCORES = 8
BLOC = B // NCORES          # 8 samples per core
BH = BLOC * HIDDEN          # 512
NPAD = 10240                # images padded to 20*512
NCHUNK = 20
CW = 512                    # n-chunk width
KC = 7                      # contraction chunks (112 each; chunk 0 has +1 bias row)
L1N = BLOC * WVEC           # 407120
L1COLS = -(-L1N // 128)     # 3181

_CACHE = {}


def _build():
    from contextlib import ExitStack
    import concourse.bass as bass
    from concourse import bacc
    import concourse.mybir as mybir
    import concourse.tile as tile

    f32 = mybir.dt.float32
    bf = mybir.dt.bfloat16
    fp8 = mybir.dt.float8e4
    AX = mybir.AxisListType.X
    OP = mybir.AluOpType
    ACT = mybir.ActivationFunctionType

    nc = bacc.Bacc("TRN2", target_bir_lowering=False, num_devices=NCORES)

    imt_d = nc.declare_dram_parameter("imt", [NCHUNK, 128, KC, CW], fp8, isOutput=False)
    w1t_d = nc.declare_dram_parameter("w1t", [128, KC, BH], fp8, isOutput=False)
    w2b_d = nc.declare_dram_parameter("w2b", [128, 4, 80], bf, isOutput=False)
    b2_d = nc.declare_dram_parameter("b2", [128, 320], bf, isOutput=False)
    oh_d = nc.declare_dram_parameter("oh", [NCHUNK, 128, 4 * 8 * 10], bf, isOutput=False)
    mask_d = nc.declare_dram_parameter("mask", [128, 32], f32, isOutput=False)
    x1_d = nc.declare_dram_parameter("x1", [128, L1COLS], bf, isOutput=False)
    t1_d = nc.declare_dram_parameter("t1", [128, L1COLS], bf, isOutput=False)
    out_d = nc.declare_dram_parameter("out", [128, 33], f32, isOutput=True)

    with tile.TileContext(nc) as tc:
        with ExitStack() as ctx:
            persist = ctx.enter_context(tc.tile_pool(name="persist", bufs=1))
            im_pool = ctx.enter_context(tc.tile_pool(name="im", bufs=4))
            oh_pool = ctx.enter_context(tc.tile_pool(name="oh", bufs=4))
            h_pool = ctx.enter_context(tc.tile_pool(name="h", bufs=3))
            s_pool = ctx.enter_context(tc.tile_pool(name="s", bufs=3))
            pa_pool = ctx.enter_context(tc.tile_pool(name="pa", bufs=4, space="PSUM"))
            pb_pool = ctx.enter_context(tc.tile_pool(name="pb", bufs=4, space="PSUM"))

            w1tA = persist.tile([128, 2, BH], fp8)
            nc.sync.dma_start(out=w1tA, in_=w1t_d[:, 0:2, :])
            w1tB = persist.tile([128, KC - 2, BH], fp8)
            nc.sync.dma_start(out=w1tB, in_=w1t_d[:, 2:KC, :])
            w2b = persist.tile([128, 4, 80], bf)
            b2 = persist.tile([128, 32, 10], bf)
            mask = persist.tile([128, 32], f32)
            acc = persist.tile([128, 32], f32)
            nc.vector.memset(acc, 0.0)
            outt = persist.tile([128, 33], f32)
            # preload the ACT table set containing exp+ln+relu+square (id 6 =
            # natural_log_exp_and_others) so no mid/tail set switch is needed
            nc.scalar.add_instruction(mybir.InstLoadActFuncSet(
                name=nc.get_next_instruction_name(), ins=[], outs=[],
                act_func_set_id=6))

            # log-sum-exp inputs collected across chunks; single Ln at the end
            # avoids per-chunk ACT table-set thrash (Exp vs Ln sets).
            ssum_all = persist.tile([128, 32, NCHUNK], f32)
            lse_all = persist.tile([128, 32, NCHUNK], f32)

            for c in range(NCHUNK):
                imtA = im_pool.tile([128, 2, CW], fp8)
                nc.sync.dma_start(out=imtA, in_=imt_d[c, :, 0:2, :])
                imtB = im_pool.tile([128, KC - 2, CW], fp8)
                nc.sync.dma_start(out=imtB, in_=imt_d[c, :, 2:KC, :])
                oht = oh_pool.tile([128, 32, 10], bf)
                nc.sync.dma_start(
                    out=oht.rearrange("p g o -> p (g o)"), in_=oh_d[c, :, :]
                )
                if c == 0:
                    nc.sync.dma_start(out=w2b, in_=w2b_d[:, :, :])
                    nc.sync.dma_start(out=b2.rearrange("p g o -> p (g o)"), in_=b2_d[:, :])
                    nc.sync.dma_start(out=mask, in_=mask_d[:, :])

                hts = [h_pool.tile([128, CW], bf, name=f"ht{j}_{c}", tag=f"ht{j}") for j in range(4)]
                for bh in range(4):
                    pa = pa_pool.tile([128, CW], f32)
                    # fp8 DoubleRow: pair k-subtiles (zero-padded rows are inert)
                    nc.tensor.matmul(
                        pa[:, :],
                        w1tA[:, :, bh * 128:(bh + 1) * 128],
                        imtA[:, :, :],
                        start=True, stop=False,
                        perf_mode=mybir.MatmulPerfMode.DoubleRow,
                    )
                    for kp in range(1, 3):
                        nc.tensor.matmul(
                            pa[:, :],
                            w1tB[:, 2 * kp - 2:2 * kp, bh * 128:(bh + 1) * 128],
                            imtB[:, 2 * kp - 2:2 * kp, :],
                            start=False, stop=False,
                            perf_mode=mybir.MatmulPerfMode.DoubleRow,
                        )
                    nc.tensor.matmul(
                        pa[:, :],
                        w1tB[0:112, 4, bh * 128:(bh + 1) * 128],
                        imtB[0:112, 4, :],
                        start=False, stop=True,
                    )
                    nc.scalar.activation(out=hts[bh], in_=pa[:, :], func=ACT.Relu)

                pb = pb_pool.tile([128, 32, 10], f32)
                for ns in range(4):
                    outap = pb[:, ns * 8:(ns + 1) * 8, :].rearrange("p g o -> p (g o)")
                    for j in range(4):
                        nc.tensor.matmul(
                            outap,
                            hts[j][:, ns * 128:(ns + 1) * 128],
                            w2b[:, j, :],
                            start=(j == 0), stop=(j == 3),
                        )

                P2 = s_pool.tile([128, 32, 10], f32)
                nc.vector.tensor_tensor(P2, pb, b2, OP.add)
                mx = s_pool.tile([128, 32], f32)
                nc.vector.tensor_reduce(out=mx, in_=P2, axis=AX, op=OP.max)
                S = s_pool.tile([128, 32, 10], f32)
                nc.vector.tensor_tensor(
                    S, P2, mx[:, :, None].broadcast_to([128, 32, 10]), OP.subtract
                )
                E = s_pool.tile([128, 32, 10], f32)
                nc.scalar.activation(out=E, in_=S, func=ACT.Exp)
                nc.vector.tensor_reduce(out=ssum_all[:, :, c], in_=E, axis=AX, op=OP.add)
                prod = s_pool.tile([128, 32, 10], f32)
                nc.vector.tensor_tensor(prod, S, oht, OP.mult)
                dotv = s_pool.tile([128, 32], f32)
                nc.vector.tensor_reduce(out=dotv, in_=prod, axis=AX, op=OP.add)
                nc.vector.tensor_add(acc, acc, dotv)
                if c == NCHUNK - 2:
                    # combined table set is resident: no switch cost here
                    nc.scalar.activation(
                        out=lse_all[:, :, 0:NCHUNK - 1],
                        in_=ssum_all[:, :, 0:NCHUNK - 1], func=ACT.Ln,
                    )

                if c == 3:
                    x1 = persist.tile([128, L1COLS], bf)
                    nc.sync.dma_start(out=x1, in_=x1_d[:, :])
                    t1 = persist.tile([128, L1COLS], bf)
                    nc.sync.dma_start(out=t1, in_=t1_d[:, :])
                if c == 6:
                    nc.vector.tensor_sub(x1, x1, t1)
                    nc.scalar.activation(out=t1, in_=x1, func=ACT.Square)
                    nc.vector.tensor_reduce(out=outt[:, 32:33], in_=t1, axis=AX, op=OP.add)

            # tail: only the last chunk's lse remains
            nc.scalar.activation(
                out=lse_all[:, :, NCHUNK - 1], in_=ssum_all[:, :, NCHUNK - 1],
                func=ACT.Ln,
            )
            nc.vector.tensor_mul(lse_all[:, :, NCHUNK - 1], lse_all[:, :, NCHUNK - 1], mask)
            lsum = persist.tile([128, 32], f32)
            nc.vector.tensor_reduce(out=lsum, in_=lse_all, axis=AX, op=OP.add)
            nc.vector.tensor_sub(outt[:, 0:32], lsum, acc)
            nc.sync.dma_start(out=out_d[:, :], in_=outt)

    nc.compile()
    return nc


def _prep_shared(images):
    """imt [NCHUNK, 113, KC, CW] bf16 (chunk-major so each chunk is one
    contiguous 810KB slab -> DMA sprays across all 16 engines):
    imagesT in 112-row chunks + ones/zeros bias row."""
    imt = np.zeros((128, KC, NPAD), dtype=np.float32)
    a = images.T.reshape(KC, 112, NTEST).transpose(1, 0, 2)  # [112, KC, NTEST]
    imt[:112, :, :NTEST] = a
    imt[112, 0, :] = 1.0
    imt = imt.reshape(128, KC, NCHUNK, CW).transpose(2, 0, 1, 3)
    return np.ascontiguousarray(imt.astype(FP8))


def _prep_core(inp1, tar1, inp2, tar2):
    """Per-core input dict from this core's 8-sample slices."""
    o1 = INPUT * HIDDEN
    o2 = o1 + HIDDEN
    o3 = o2 + HIDDEN * OUT
    W1 = inp2[:, :o1].reshape(BLOC, HIDDEN, INPUT)
    B1 = inp2[:, o1:o2].reshape(BH)
    W2 = inp2[:, o2:o3].reshape(BLOC, OUT, HIDDEN)
    B2 = inp2[:, o3:].reshape(1, BLOC * OUT)

    w1t = np.zeros((128, KC, BH), dtype=np.float32)
    # W1 [b,h,d] -> [d, b*64+h] -> chunks [112, KC, BH]
    w1t[:112] = W1.reshape(BH, KC, 112).transpose(2, 1, 0)
    w1t[112, 0, :] = B1

    w2blk = np.zeros((BH, BLOC * OUT), dtype=np.float32)
    for b in range(BLOC):
        w2blk[b * HIDDEN:(b + 1) * HIDDEN, b * OUT:(b + 1) * OUT] = W2[b].T
    w2b = w2blk.reshape(4, 128, 80).transpose(1, 0, 2)

    # one-hot labels in device layout [NCHUNK, 128, 4*8*10]
    oh = np.zeros((BLOC, NPAD, OUT), dtype=np.float32)
    oh[np.arange(BLOC)[:, None], np.arange(NTEST)[None, :], tar2.astype(np.int64)] = 1.0
    # [b, chunk, ns, p, o] -> [chunk, p, ns, b, o]
    ohd = oh.reshape(BLOC, NCHUNK, 4, 128, OUT).transpose(1, 3, 2, 0, 4)
    ohd = ohd.reshape(NCHUNK, 128, 4 * BLOC * OUT)

    mask = np.zeros((128, 32), dtype=np.float32)
    n0 = (NCHUNK - 1) * CW
    for ns in range(4):
        valid = np.clip(NTEST - (n0 + ns * 128), 0, 128)
        mask[:valid, ns * 8:(ns + 1) * 8] = 1.0

    x1 = np.zeros((128 * L1COLS,), dtype=np.float32)
    x1[:L1N] = inp1.ravel()
    t1 = np.zeros((128 * L1COLS,), dtype=np.float32)
    t1[:L1N] = tar1.ravel()

    return {
        "w1t": np.ascontiguousarray(w1t.astype(FP8)),
        "w2b": np.ascontiguousarray(w2b.astype(BF16)),
        "b2": np.ascontiguousarray(np.tile(B2.reshape(-1), (128, 4)).astype(BF16)),
        "oh": np.ascontiguousarray(ohd.astype(BF16)),
        "mask": mask,
        "x1": x1.reshape(128, L1COLS).astype(BF16),
        "t1": t1.reshape(128, L1COLS).astype(BF16),
    }


def kernel(inp1, tar1, inp2, tar2, images, _want_results=False):
    from concourse.bass_utils import run_bass_kernel_spmd

    inp1 = np.asarray(inp1, dtype=np.float32)
    tar1 = np.asarray(tar1, dtype=np.float32)
    inp2 = np.asarray(inp2, dtype=np.float32)
    tar2 = np.asarray(tar2)
    images = np.asarray(images, dtype=np.float32)

    if "nc" not in _CACHE:
        _CACHE["nc"] = _build()
    nc = _CACHE["nc"]

    imt = _prep_shared(images)
    in_maps = []
    for core in range(NCORES):
        s = slice(core * BLOC, (core + 1) * BLOC)
        m = _prep_core(inp1[s], tar1[s], inp2[s], tar2[s])
        m["imt"] = imt
        in_maps.append(m)

    res = run_bass_kernel_spmd(nc, in_maps, core_ids=list(range(NCORES)))

    ce_sum = 0.0
    sq_sum = 0.0
    for core in range(NCORES):
        o = res.results[core]["out"].astype(np.float64)
        ce_sum += np.sum(o[:, 0:32])
        sq_sum += np.sum(o[:, 32])

    loss1 = 20.0 * sq_sum / (B * WVEC)
    loss2 = ce_sum / (B * NTEST)
    combined = loss1 + loss2
    out = (
        np.float32(combined),
        np.float32(loss1),
        np.float32(loss2),
    )
    if _want_results:
        return out, res
    return out



# revision 9
# speedup vs baseline: 1.0017x; 1.0017x over previous
"""Trainium2 Bass kernel for nn_CustomerizedLoss (MSE + per-sample weight-conditioned
MLP cross-entropy over a fixed image set).

Sharding: model-batch dim B=64 split across 8 NeuronCores (8 samples each);
the 10000x784 image matrix is replicated (shipped transposed, fp8).

Per core, per 512-image chunk c:
  mm1:  h^T[bh=512, n=512] = W1T^T @ imagesT  as 3 full fp8-DoubleRow matmuls
        (k=256 each, covering features 0..768) + a k=17 tail (features 768..784
        + bias row) issued as 4 row-tiled matmuls (tile_position=(32b,0)) that
        run concurrently on the PE array.
  relu: 3 on ScalarE + 1 on VectorE, psum->sbuf bf16
  mm2:  logits[n, 80] = h^T^T @ W2blk (block-diag W2), lagged one chunk in the
        PE stream so it never waits on the relus.
  CE:   grouped (4ns x 8samples, 10-way) log-softmax pieces: bias-add + max on
        VectorE (bf16), subtract+onehot-dot on GpSimd, exp on ScalarE;
        per-chunk ln(sum exp) deferred; single Ln over chunks 0..18 at c==18.
  loss1: (inp1-tar1)^2 in 8 slices: subtract on VectorE, Square+accum_out on
        ScalarE, spread over chunks 9..16.
Host combines the [128, 9] per-core partial sums into (combined, loss1, loss2).
"""

import numpy as np
import ml_dtypes

BF16 = ml_dtypes.bfloat16
FP8 = ml_dtypes.float8_e4m3

INPUT, HIDDEN, OUT = 784, 64, 10
NTEST, B, WVEC = 10000, 64, 50890
NCORES = 8
BLOC = B // NCORES          # 8 samples per core
BH = BLOC * HIDDEN          # 512
NPAD = 10240                # images padded to 20*512
NCHUNK = 20
CW = 512                    # n-chunk width
KM = 6                      # main contraction subtiles (128 rows each = 768)
KT = 17                     # tail rows: 16 leftover features + bias row
L1N = BLOC * WVEC           # 407120
L1COLS = -(-L1N // 128)     # 3181

_CACHE = {}


def _build():
    from contextlib import ExitStack
    import concourse.bass as bass
    from concourse import bacc
    import concourse.mybir as mybir
    import concourse.tile as tile

    f32 = mybir.dt.float32
    bf = mybir.dt.bfloat16
    fp8 = mybir.dt.float8e4
    AX = mybir.AxisListType.X
    OP = mybir.AluOpType
    ACT = mybir.ActivationFunctionType
    DR = mybir.MatmulPerfMode.DoubleRow

    nc = bacc.Bacc("TRN2", target_bir_lowering=False, num_devices=NCORES)

    imt_d = nc.declare_dram_parameter("imt", [NCHUNK, 128, KM, CW], fp8, isOutput=False)
    im2_d = nc.declare_dram_parameter("imt2", [NCHUNK, 128, CW], fp8, isOutput=False)
    w1t_d = nc.declare_dram_parameter("w1t", [128, KM, BH], fp8, isOutput=False)
    w1l_d = nc.declare_dram_parameter("w1l", [128, 4, 128], fp8, isOutput=False)
    w2b_d = nc.declare_dram_parameter("w2b", [128, 4, 80], bf, isOutput=False)
    b2_d = nc.declare_dram_parameter("b2", [128, 320], bf, isOutput=False)
    oh_d = nc.declare_dram_parameter("oh", [NCHUNK, 128, 4 * 8 * 10], bf, isOutput=False)
    mask_d = nc.declare_dram_parameter("mask", [128, 32], f32, isOutput=False)
    x1_d = nc.declare_dram_parameter("x1", [128, L1COLS], bf, isOutput=False)
    t1_d = nc.declare_dram_parameter("t1", [128, L1COLS], bf, isOutput=False)
    out_d = nc.declare_dram_parameter("out", [128, 9], f32, isOutput=True)

    # loss1 slice widths (8 pieces over L1COLS)
    l1w = [398] * 7 + [L1COLS - 7 * 398]
    l1o = [sum(l1w[:i]) for i in range(8)]

    with tile.TileContext(nc) as tc:
        with ExitStack() as ctx:
            persist = ctx.enter_context(tc.tile_pool(name="persist", bufs=1))
            im_pool = ctx.enter_context(tc.tile_pool(name="im", bufs=3))
            oh_pool = ctx.enter_context(tc.tile_pool(name="oh", bufs=3))
            h_pool = ctx.enter_context(tc.tile_pool(name="h", bufs=3))
            s_pool = ctx.enter_context(tc.tile_pool(name="s", bufs=3))
            m_pool = ctx.enter_context(tc.tile_pool(name="mse", bufs=2))
            # PSUM is 8 banks of 2KB/partition; each pa tile is one full bank.
            # Ring depths chosen so the next chunk's matmuls never wait on a
            # relu: bh1/bh3 banks are freed first by the relu order below.
            pa_pools = [
                ctx.enter_context(tc.tile_pool(name=f"pa{b}", bufs=n, space="PSUM"))
                for b, n in enumerate([2, 1, 2, 1])
            ]
            pb_pool = ctx.enter_context(tc.tile_pool(name="pb", bufs=2, space="PSUM"))

            # highest priority DMA: what the very first matmul needs
            w1tA = persist.tile([128, 2, BH], fp8)
            nc.sync.dma_start(out=w1tA, in_=w1t_d[:, 0:2, :])

            w1tB = persist.tile([128, KM - 2, BH], fp8)
            w1l = persist.tile([128, 4, 128], fp8)
            w2b = persist.tile([128, 4, 80], bf)
            b2 = persist.tile([128, 32, 10], bf)
            mask = persist.tile([128, 32], f32)
            outt = persist.tile([128, 9], f32)
            ssum_all = persist.tile([128, NCHUNK, 32], f32)
            lse_all = persist.tile([128, NCHUNK, 32], f32)
            dv_all = persist.tile([128, NCHUNK, 32], f32)
            x1 = persist.tile([128, L1COLS], bf)
            t1 = persist.tile([128, L1COLS], bf)

            prev = None  # (hts, oht, pas-of-chunk, c) pending mm2+CE
            for c in range(NCHUNK + 1):
                if c < NCHUNK:
                    imtA = im_pool.tile([128, 2, CW], fp8, tag="imA")
                    nc.sync.dma_start(out=imtA, in_=imt_d[c, :, 0:2, :])
                    if c == 0:
                        nc.sync.dma_start(out=w1tB, in_=w1t_d[:, 2:KM, :])
                    imtB = im_pool.tile([128, KM - 2, CW], fp8, tag="imB")
                    nc.sync.dma_start(out=imtB, in_=imt_d[c, :, 2:KM, :])
                    if c == 0:
                        nc.sync.dma_start(out=w1l, in_=w1l_d[:, :, :])
                    imt2 = im_pool.tile([128, CW], fp8, tag="im2")
                    nc.sync.dma_start(out=imt2, in_=im2_d[c, :, :])
                    oht = oh_pool.tile([128, 32, 10], bf)
                    nc.sync.dma_start(
                        out=oht.rearrange("p g o -> p (g o)"), in_=oh_d[c, :, :]
                    )
                    if c == 0:
                        nc.sync.dma_start(out=w2b, in_=w2b_d[:, :, :])
                        nc.sync.dma_start(
                            out=b2.rearrange("p g o -> p (g o)"), in_=b2_d[:, :]
                        )
                        nc.sync.dma_start(out=mask, in_=mask_d[:, :])
                        # preload ACT table set 6 (exp+ln+relu+square together)
                        nc.scalar.add_instruction(mybir.InstLoadActFuncSet(
                            name=nc.get_next_instruction_name(), ins=[], outs=[],
                            act_func_set_id=6))
                    if c == 4:
                        nc.sync.dma_start(out=x1[:, 0:1591], in_=x1_d[:, 0:1591])
                    if c == 5:
                        nc.sync.dma_start(out=x1[:, 1591:L1COLS], in_=x1_d[:, 1591:L1COLS])
                    if c == 6:
                        nc.sync.dma_start(out=t1[:, 0:1591], in_=t1_d[:, 0:1591])
                    if c == 7:
                        nc.sync.dma_start(out=t1[:, 1591:L1COLS], in_=t1_d[:, 1591:L1COLS])

                    # ---- PE: mm1 for chunk c ----
                    pas = [pa_pools[b].tile([128, CW], f32, name=f"pa{b}_{c}",
                                            tag=f"pa{b}")
                           for b in range(4)]
                    for b in range(4):
                        for kp in range(3):
                            wsrc = w1tA if kp == 0 else w1tB
                            isrc = imtA if kp == 0 else imtB
                            ks = slice(0, 2) if kp == 0 else slice(2 * kp - 2, 2 * kp)
                            nc.tensor.matmul(
                                pas[b][:, :],
                                wsrc[:, ks, b * 128:(b + 1) * 128],
                                isrc[:, ks, :],
                                start=(kp == 0), stop=False,
                                perf_mode=DR,
                            )
                    for b in range(4):
                        nc.tensor.matmul(
                            pas[b][:, :],
                            w1l[32 * b:32 * b + KT, b, :],
                            imt2[32 * b:32 * b + KT, :],
                            start=False, stop=True,
                            tile_position=(32 * b, 0),
                        )

                # ---- PE: mm2 for chunk c-1 (lagged so relus never stall PE) ----
                if prev is not None:
                    phts, poht, pc = prev
                    pb = pb_pool.tile([128, 32, 10], f32)
                    for ns in range(4):
                        outap = pb[:, ns * 8:(ns + 1) * 8, :].rearrange("p g o -> p (g o)")
                        for j in range(4):
                            nc.tensor.matmul(
                                outap,
                                phts[j][:, ns * 128:(ns + 1) * 128],
                                w2b[:, j, :],
                                start=(j == 0), stop=(j == 3),
                            )
                    # ---- CE chain for chunk c-1 ----
                    P2 = s_pool.tile([128, 32, 10], f32, tag="P2")
                    nc.vector.tensor_tensor(P2, pb, b2, OP.add)
                    mx = s_pool.tile([128, 32], f32, tag="mx")
                    nc.vector.tensor_reduce(out=mx, in_=P2, axis=AX, op=OP.max)
                    S = s_pool.tile([128, 32, 10], f32, tag="S")
                    nc.vector.tensor_tensor(
                        S, P2, mx[:, :, None].broadcast_to([128, 32, 10]), OP.subtract
                    )
                    E = s_pool.tile([128, 32, 10], f32, tag="E")
                    nc.scalar.activation(out=E, in_=S, func=ACT.Exp)
                    nc.vector.tensor_reduce(
                        out=ssum_all[:, pc, :], in_=E, axis=AX, op=OP.add
                    )
                    prod = s_pool.tile([128, 32, 10], bf, tag="prod")
                    nc.gpsimd.tensor_tensor(prod, S, poht, OP.mult)
                    nc.vector.tensor_reduce(
                        out=dv_all[:, pc, :], in_=prod, axis=AX, op=OP.add
                    )

                if c < NCHUNK:
                    # ---- relus for chunk c ----
                    # order matters: bh1/bh3 banks (ring depth 1) are drained
                    # first on ScalarE; bh0 drains on VectorE in parallel.
                    hts = [h_pool.tile([128, CW], bf, name=f"ht{j}_{c}", tag=f"ht{j}")
                           for j in range(4)]
                    nc.vector.tensor_scalar_max(hts[0], pas[0], 0.0)
                    nc.scalar.activation(out=hts[1], in_=pas[1], func=ACT.Relu)
                    nc.scalar.activation(out=hts[3], in_=pas[3], func=ACT.Relu)
                    nc.scalar.activation(out=hts[2], in_=pas[2], func=ACT.Relu)

                    # ---- loss1 pieces on chunks 9..16 ----
                    if 9 <= c <= 16:
                        i = c - 9
                        o, w = l1o[i], l1w[i]
                        d = m_pool.tile([128, w], bf, name=f"d{i}", tag="msed")
                        nc.vector.tensor_tensor(d, x1[:, o:o + w], t1[:, o:o + w],
                                                OP.subtract)
                        sq = m_pool.tile([128, w], bf, name=f"sq{i}", tag="msq")
                        nc.scalar.activation(out=sq, in_=d, func=ACT.Square,
                                             accum_out=outt[:, 1 + i:2 + i])

                    if c == 18:
                        # chunks 0..17 are complete by now (CE is lagged one
                        # chunk); 18/19 are handled in the tail
                        nc.scalar.activation(
                            out=lse_all[:, 0:NCHUNK - 2, :],
                            in_=ssum_all[:, 0:NCHUNK - 2, :], func=ACT.Ln,
                        )
                    prev = (hts, oht, c)

            # ---- tail: last two chunks' lse + final reductions ----
            nc.scalar.activation(
                out=lse_all[:, NCHUNK - 2:NCHUNK, :],
                in_=ssum_all[:, NCHUNK - 2:NCHUNK, :], func=ACT.Ln,
            )
            nc.vector.tensor_mul(lse_all[:, NCHUNK - 1, :],
                                 lse_all[:, NCHUNK - 1, :], mask)
            lsum = persist.tile([128, 1], f32)
            nc.vector.tensor_reduce(
                out=lsum, in_=lse_all.rearrange("p a b -> p (a b)"), axis=AX, op=OP.add
            )
            dvs = persist.tile([128, 1], f32)
            nc.vector.tensor_reduce(
                out=dvs, in_=dv_all.rearrange("p a b -> p (a b)"), axis=AX, op=OP.add
            )
            nc.vector.tensor_sub(outt[:, 0:1], lsum, dvs)
            nc.sync.dma_start(out=out_d[:, :], in_=outt)

    nc.compile()
    return nc


def _prep_shared(images):
    """Shared (replicated) device arrays from the image matrix.

    imt  [NCHUNK, 128, KM, CW] fp8: imagesT features 0..768 in 128-row
         subtiles, chunk-major so each chunk is one contiguous slab.
    imt2 [NCHUNK, 128, CW] fp8: features 768..784 + ones(bias) row,
         replicated at base partitions 0/32/64/96 for row-tiled matmuls.
    """
    imT = np.zeros((INPUT, NPAD), dtype=np.float32)
    imT[:, :NTEST] = images.T
    a = imT[:768].reshape(KM, 128, NCHUNK, CW)
    imt = a.transpose(2, 1, 0, 3)  # [NCHUNK, 128, KM, CW]

    imt2 = np.zeros((NCHUNK, 128, CW), dtype=np.float32)
    t = imT[768:784].reshape(16, NCHUNK, CW).transpose(1, 0, 2)  # [NCHUNK,16,CW]
    for r in range(4):
        imt2[:, 32 * r:32 * r + 16, :] = t
        imt2[:, 32 * r + 16, :] = 1.0
    return {
        "imt": np.ascontiguousarray(imt.astype(FP8)),
        "imt2": np.ascontiguousarray(imt2.astype(FP8)),
    }


def _prep_core(inp1, tar1, inp2, tar2):
    """Per-core input dict from this core's 8-sample slices."""
    o1 = INPUT * HIDDEN
    o2 = o1 + HIDDEN
    o3 = o2 + HIDDEN * OUT
    W1 = inp2[:, :o1].reshape(BLOC, HIDDEN, INPUT)
    B1 = inp2[:, o1:o2].reshape(BH)
    W2 = inp2[:, o2:o3].reshape(BLOC, OUT, HIDDEN)
    B2 = inp2[:, o3:].reshape(1, BLOC * OUT)

    W = W1.reshape(BH, INPUT)  # bh = b*64+h
    w1t = W[:, :768].T.reshape(KM, 128, BH).transpose(1, 0, 2)  # [128, KM, BH]

    w1l = np.zeros((128, 4, 128), dtype=np.float32)
    for b4 in range(4):
        w1l[32 * b4:32 * b4 + 16, b4, :] = W[b4 * 128:(b4 + 1) * 128, 768:784].T
        w1l[32 * b4 + 16, b4, :] = B1[b4 * 128:(b4 + 1) * 128]

    w2blk = np.zeros((BH, BLOC * OUT), dtype=np.float32)
    for b in range(BLOC):
        w2blk[b * HIDDEN:(b + 1) * HIDDEN, b * OUT:(b + 1) * OUT] = W2[b].T
    w2b = w2blk.reshape(4, 128, 80).transpose(1, 0, 2)

    # one-hot labels in device layout [NCHUNK, 128, 4*8*10]
    oh = np.zeros((BLOC, NPAD, OUT), dtype=np.float32)
    oh[np.arange(BLOC)[:, None], np.arange(NTEST)[None, :], tar2.astype(np.int64)] = 1.0
    # [b, chunk, ns, p, o] -> [chunk, p, ns, b, o]
    ohd = oh.reshape(BLOC, NCHUNK, 4, 128, OUT).transpose(1, 3, 2, 0, 4)
    ohd = ohd.reshape(NCHUNK, 128, 4 * BLOC * OUT)

    mask = np.zeros((128, 32), dtype=np.float32)
    n0 = (NCHUNK - 1) * CW
    for ns in range(4):
        valid = np.clip(NTEST - (n0 + ns * 128), 0, 128)
        mask[:valid, ns * 8:(ns + 1) * 8] = 1.0

    x1 = np.zeros((128 * L1COLS,), dtype=np.float32)
    x1[:L1N] = inp1.ravel()
    t1 = np.zeros((128 * L1COLS,), dtype=np.float32)
    t1[:L1N] = tar1.ravel()

    return {
        "w1t": np.ascontiguousarray(w1t.astype(FP8)),
        "w1l": np.ascontiguousarray(w1l.astype(FP8)),
        "w2b": np.ascontiguousarray(w2b.astype(BF16)),
        "b2": np.ascontiguousarray(np.tile(B2.reshape(-1), (128, 4)).astype(BF16)),
        "oh": np.ascontiguousarray(ohd.astype(BF16)),
        "mask": mask,
        "x1": x1.reshape(128, L1COLS).astype(BF16),
        "t1": t1.reshape(128, L1COLS).astype(BF16),
    }


def kernel(inp1, tar1, inp2, tar2, images, _want_results=False):
    from concourse.bass_utils import run_bass_kernel_spmd

    inp1 = np.asarray(inp1, dtype=np.float32)
    tar1 = np.asarray(tar1, dtype=np.float32)
    inp2 = np.asarray(inp2, dtype=np.float32)
    tar2 = np.asarray(tar2)
    images = np.asarray(images, dtype=np.float32)

    if "nc" not in _CACHE:
        _CACHE["nc"] = _build()
    nc = _CACHE["nc"]

    shared = _prep_shared(images)
    in_maps = []
    for core in range(NCORES):
        s = slice(core * BLOC, (core + 1) * BLOC)
        m = _prep_core(inp1[s], tar1[s], inp2[s], tar2[s])
        m.update(shared)
        in_maps.append(m)

    res = run_bass_kernel_spmd(nc, in_maps, core_ids=list(range(NCORES)))

    ce_sum = 0.0
    sq_sum = 0.0
    for core in range(NCORES):
        o = res.results[core]["out"].astype(np.float64)
        ce_sum += np.sum(o[:, 0])
        sq_sum += np.sum(o[:, 1:9])

    loss1 = 20.0 * sq_sum / (B * WVEC)
    loss2 = ce_sum / (B * NTEST)
    combined = loss1 + loss2
    out = (
        np.float32(combined),
        np.float32(loss1),
        np.float32(loss2),
    )
    if _want_results:
        return out, res
    return out
